# revision 9
# baseline (speedup 1.0000x reference)
"""GCNConv (COO SpMM aggregation + dense GEMM) on 8 Trainium2 NeuronCores.

  msgs = edge_vals[:, None] * x[edge_col]          # [E, 64] gather+scale
  agg  = segment_sum(msgs, edge_row, N)            # [N, 64] scatter-add
  out  = agg @ weight                              # [N, 64] GEMM

Sharding: destination-node sharding (each core owns a contiguous row range
and all edges targeting it) -> zero collectives.  Host-side index prep sorts
edges by (core, 128-row dest block, x-quarter) and pads each (block, quarter)
group to a multiple of 128 edges.

Per core:
  - gpsimd.dma_gather pulls the 64-float source rows (256B each) from HBM
    (per-edge descriptors; desc-gen on the Q7s is the throughput limit)
  - one DVE tensor_tensor per (supergroup, quarter) scales msgs by edge_vals
    (broadcast AP along the feature dim)
  - one DVE tensor_tensor per dest block builds the one-hot
    oh[e, r] = (dest[e] == r) via iota-compare with a broadcast dest AP
  - TensorE per 128-edge chunk: psum[128 rows, 64] += oh.T @ msgs
    (segment-sum as matmul)
  - per 128-row block: PE-transpose agg -> aggT, then outT[64,128] = W.T@aggT
  - one contiguous [64, rows] output DMA; host concatenates + transposes.
"""

import os
import sys

import numpy as np

if "/opt/trn_rl_repo" not in sys.path:
    sys.path.insert(0, "/opt/trn_rl_repo")

# ---------------------------------------------------------------- constants
N = 100000
E = 1600000
D = 64
CORES = 8
RPC = 12544          # rows per core (= BLOCKS * 128; 8*12544 = 100352 >= N)
BLOCKS = RPC // 128  # 98 dest blocks per core
Q = 4                # x row-table quarters (int16 gather index limit)
XQ = 25088           # rows per quarter (4*25088 = 100352)
G = 1                # one gather call per (dest block, quarter)
NGROUPS = BLOCKS // G

LAST_EXEC_TIME_NS = None
_CACHE = {}


# ---------------------------------------------------------------- host prep
def _prep(x, weight, edge_vals, edge_row, edge_col):
    """Sort/pad edges; build per-core gather-index / dest / val arrays."""
    e_row = np.asarray(edge_row, dtype=np.int64)
    e_col = np.asarray(edge_col, dtype=np.int64)
    ev = np.asarray(edge_vals, dtype=np.float32)
    x = np.asarray(x, dtype=np.float32)
    weight = np.asarray(weight, dtype=np.float32)
    ne = e_row.shape[0]

    core = e_row // RPC
    blk = (e_row % RPC) // 128
    dest = (e_row % 128).astype(np.float32)
    qq = e_col // XQ
    lidx = (e_col - qq * XQ).astype(np.int16)

    NG = CORES * BLOCKS * Q
    gkey = (core * BLOCKS + blk) * Q + qq
    order = np.argsort(gkey, kind="stable")
    counts = np.bincount(gkey, minlength=NG)
    Cq = max(1, int(-(-counts.max() // 128)))  # chunks per (block, quarter)
    SL = Cq * 128
    starts = np.zeros(NG, np.int64)
    starts[1:] = np.cumsum(counts)[:-1]
    gs = gkey[order]
    slot = gs * SL + (np.arange(ne, dtype=np.int64) - starts[gs])

    # Each (block, quarter) gather call is trimmed to a per-call STATIC
    # count = max real count over the 8 cores (num_idxs_reg must equal the
    # post-trim count on every core, and it is baked into the program).
    # Slots [real, cnt) pad with idx=0 (descriptor-generating, harmless);
    # slots [cnt, SL) pad with idx=-1 (trimmed by the ucode: no descriptor,
    # slot left stale -> zeroed by the val=0 scale op).  dest=-1 never
    # matches the iota compare; val=0.
    idx_flat = np.full(NG * SL, -1, np.int16)
    dst_flat = np.full(NG * SL, -1.0, np.float32)
    val_flat = np.zeros(NG * SL, np.float32)
    idx_flat[slot] = lidx[order]
    dst_flat[slot] = dest[order]
    val_flat[slot] = ev[order]

    assert G == 1
    cnts = counts.reshape(CORES, BLOCKS, Q).max(axis=0)  # [BLOCKS, Q]
    idx4 = idx_flat.reshape(CORES, BLOCKS, Q, SL)
    fillmask = (np.arange(SL)[None, None, :] < cnts[:, :, None]) & (
        idx4 == -1
    )
    idx4[fillmask] = 0

    CALLE = G * SL                   # edges per dma_gather call
    S16 = CALLE // 16

    def to_calls(a):
        # [CORES, BLOCKS, Q, SL] -> [CORES, NGROUPS, Q, G*SL] call-major
        a = a.reshape(CORES, NGROUPS, G, Q, SL)
        return np.ascontiguousarray(a.transpose(0, 1, 3, 2, 4)).reshape(
            CORES, NGROUPS, Q, CALLE
        )

    idx_c = to_calls(idx_flat)
    dst_c = to_calls(dst_flat)
    val_c = to_calls(val_flat)

    # gather idx wrap: idx for call-edge i lives at [i % 16, i // 16],
    # replicated across the 8 gpsimd cores -> [128, Q, CALLE//16]
    w16 = idx_c.reshape(CORES, NGROUPS, Q, S16, 16)
    w16 = np.moveaxis(w16, 4, 2)                      # [C, NGR, 16, Q, S16]
    gidx = np.ascontiguousarray(np.tile(w16, (1, 1, 8, 1, 1)))

    # dest/val layout matches gather output: [p, q, j] = call-edge j*128+p
    def to_pj(a):
        a = a.reshape(CORES, NGROUPS, Q, G * Cq, 128)
        return np.ascontiguousarray(np.moveaxis(a, 4, 2))  # [C,NGR,128,Q,G*Cq]

    gdst = to_pj(dst_c)
    gval = to_pj(val_c)

    x_pad = np.zeros((Q * XQ, D), np.float32)
    x_pad[:N] = x
    # iota replicated per chunk-slot: irep[p, s, m] = m
    irep = np.broadcast_to(
        np.arange(128, dtype=np.float32), (128, Q * Cq, 128)
    ).copy()
    ident = np.eye(128, dtype=np.float32)

    in_maps = []
    for k in range(CORES):
        in_maps.append(
            {
                "xq": x_pad,
                "w": np.ascontiguousarray(weight),
                "irep": irep,
                "ident": ident,
                "gidx": gidx[k],
                "gdst": gdst[k],
                "gval": gval[k],
            }
        )
    return in_maps, Cq, cnts


# ------------------------------------------------------------- bass program
def _build(Cq, counts):
    import concourse.bacc as bacc
    import concourse.mybir as mybir
    import concourse.tile as tile

    f32 = mybir.dt.float32
    i16 = mybir.dt.int16
    SL = Cq * 128
    CALLE = G * SL
    S16 = CALLE // 16
    JQ = G * Cq          # msgs slots per (call, quarter)
    NCH = Q * Cq         # chunk-slots per block

    nc = bacc.Bacc(
        "TRN2", target_bir_lowering=False, debug=False, num_devices=CORES
    )
    x_d = nc.dram_tensor("xq", [Q * XQ, D], f32, kind="ExternalInput")
    w_d = nc.dram_tensor("w", [D, D], f32, kind="ExternalInput")
    irep_d = nc.dram_tensor("irep", [128, NCH, 128], f32, kind="ExternalInput")
    id_d = nc.dram_tensor("ident", [128, 128], f32, kind="ExternalInput")
    gidx_d = nc.dram_tensor("gidx", [NGROUPS, 128, Q, S16], i16, kind="ExternalInput")
    gdst_d = nc.dram_tensor("gdst", [NGROUPS, 128, Q, JQ], f32, kind="ExternalInput")
    gval_d = nc.dram_tensor("gval", [NGROUPS, 128, Q, JQ], f32, kind="ExternalInput")
    outT_d = nc.dram_tensor("outT", [D, RPC], f32, kind="ExternalOutput")

    eq = mybir.AluOpType.is_equal
    mul = mybir.AluOpType.mult

    with tile.TileContext(nc) as tc:
        with (
            tc.tile_pool(name="const", bufs=1) as cpool,
            tc.tile_pool(name="io", bufs=3) as iopool,
            tc.tile_pool(name="vh", bufs=3) as vhpool,
            tc.tile_pool(name="sb", bufs=4) as sbpool,
            tc.tile_pool(name="outsb", bufs=1) as opool,
            tc.tile_pool(name="pa", bufs=3, space="PSUM") as papool,
            tc.tile_pool(name="pt", bufs=2, space="PSUM") as ptpool,
            tc.tile_pool(name="po", bufs=2, space="PSUM") as popool,
        ):
            w_sb = cpool.tile([D, D], f32, name="w_sb")
            irep_sb = cpool.tile([128, NCH, 128], f32, name="irep_sb")
            id_sb = cpool.tile([128, 128], f32, name="id_sb")
            outT_sb = opool.tile([D, RPC], f32, name="outT_sb")
            nc.sync.dma_start(out=w_sb[:], in_=w_d[:])
            nc.sync.dma_start(out=irep_sb[:], in_=irep_d[:])
            nc.sync.dma_start(out=id_sb[:], in_=id_d[:])

            # persistent msgs tiles, manually ping-ponged: gather calls with
            # trimmed -1 tails leave slots untouched, so each tile is zeroed
            # once here and thereafter always holds finite data (the val=0
            # scale op rewrites stale slots to 0 every round).
            NB = 3
            msgs_t = [
                [
                    cpool.tile([128, JQ, D], f32, name=f"msgs{bi}_{q}")
                    for q in range(Q)
                ]
                for bi in range(NB)
            ]
            for row in msgs_t:
                for t in row:
                    nc.vector.memset(t[:], 0.0)

            for g in range(NGROUPS):
                idx_t = iopool.tile([128, Q, S16], i16, tag="idx", name=f"idx{g}")
                dst_t = iopool.tile([128, Q, JQ], f32, tag="dst", name=f"dst{g}")
                val_t = iopool.tile([128, Q, JQ], f32, tag="val", name=f"val{g}")
                nc.sync.dma_start(out=idx_t[:], in_=gidx_d[g])
                nc.sync.dma_start(out=dst_t[:], in_=gdst_d[g])
                nc.sync.dma_start(out=val_t[:], in_=gval_d[g])

                msgs = msgs_t[g % NB]
                for q in range(Q):
                    m = msgs[q]
                    nidx_reg = int(counts[g][q])
                    nc.gpsimd.dma_gather(
                        m[:],
                        x_d[q * XQ : (q + 1) * XQ, :],
                        idx_t[:, q, :],
                        CALLE,
                        nidx_reg,
                        D,
                        # default single_packet=True needs the whole call in
                        # the 1024-desc SWDGE ring -> device crash at 4480
                        single_packet=False,
                    )
                    # scale msgs by edge_vals (broadcast along features)
                    nc.vector.tensor_tensor(
                        m[:],
                        m[:],
                        val_t[:, q, :].unsqueeze(2).broadcast_to([128, JQ, D]),
                        mul,
                    )

                for lb in range(G):
                    b = g * G + lb
                    # one-hot for the whole block in one DVE op:
                    # vh[p, q, c, m] = (dest[p, q, lb*Cq+c] == m)
                    vh = vhpool.tile([128, Q, Cq, 128], f32, tag="vh", name=f"vh{b}")
                    nc.vector.tensor_tensor(
                        vh[:],
                        irep_sb[:].rearrange("p (q c) m -> p q c m", q=Q),
                        dst_t[:, :, lb * Cq : (lb + 1) * Cq]
                        .unsqueeze(3)
                        .broadcast_to([128, Q, Cq, 128]),
                        eq,
                    )
                    pa = papool.tile([128, D], f32, tag="pa", name=f"pa{b}")
                    nmm = Q * Cq
                    i = 0
                    for q in range(Q):
                        for c in range(Cq):
                            j = lb * Cq + c
                            nc.tensor.matmul(
                                pa[:],
                                vh[:, q, c, :],
                                msgs[q][:, j, :],
                                start=(i == 0),
                                stop=(i == nmm - 1),
                            )
                            i += 1
                    agg_sb = sbpool.tile([128, D], f32, tag="agg", name=f"agg{b}")
                    nc.vector.tensor_copy(agg_sb[:], pa[:])
                    pt = ptpool.tile([D, 128], f32, tag="pt", name=f"pt{b}")
                    nc.tensor.transpose(pt[:], agg_sb[:], id_sb[:])
                    aggT_sb = sbpool.tile([D, 128], f32, tag="aggT", name=f"aggT{b}")
                    nc.vector.tensor_copy(aggT_sb[:], pt[:])
                    po = popool.tile([D, 128], f32, tag="po", name=f"po{b}")
                    nc.tensor.matmul(po[:], w_sb[:], aggT_sb[:], start=True, stop=True)
                    nc.vector.tensor_copy(
                        outT_sb[:, b * 128 : (b + 1) * 128], po[:]
                    )

            nc.sync.dma_start(out=outT_d[:], in_=outT_sb[:])

    nc.compile()
    return nc


# ----------------------------------------------------------------- kernel()
def _ensure_ntff_hook():
    """Provide antenv.axon_hooks (absent in this image) so that
    run_bass_kernel_spmd's BASS_TRACE path can register the axon NTFF
    profiler instead of crashing on import."""
    try:
        import antenv.axon_hooks  # noqa: F401

        return
    except ImportError:
        pass
    import types

    import antenv

    mod = types.ModuleType("antenv.axon_hooks")
    holder = {"hook": None}
    mod.set_axon_ntff_profile_hook = lambda h: holder.__setitem__("hook", h)
    mod.get_axon_ntff_profile_hook = lambda: holder["hook"]
    sys.modules["antenv.axon_hooks"] = mod
    antenv.axon_hooks = mod
    try:
        from trn_agent_boot.trn_boot import _ntff_profile_via_ctypes

        mod.set_axon_ntff_profile_hook(
            _ntff_profile_via_ctypes("/opt/axon/libaxon_pjrt.so")
        )
    except Exception:
        pass


def kernel(x, weight, edge_vals, edge_row, edge_col):
    global LAST_EXEC_TIME_NS
    from concourse.bass_utils import run_bass_kernel_spmd

    if os.environ.get("BASS_TRACE"):
        _ensure_ntff_hook()

    in_maps, Cq, cnts = _prep(x, weight, edge_vals, edge_row, edge_col)
    key = (Cq, cnts.tobytes())
    if key not in _CACHE:
        _CACHE[key] = _build(Cq, cnts)
    nc = _CACHE[key]

    res = run_bass_kernel_spmd(nc, in_maps, list(range(CORES)))
    LAST_EXEC_TIME_NS = res.exec_time_ns

    outT = np.concatenate([res.results[k]["outT"] for k in range(CORES)], axis=1)
    out = np.ascontiguousarray(outT.T[:N])
    return out.astype(np.float32, copy=False)


# revision 11
# speedup vs baseline: 1.3467x; 1.3467x over previous
"""GCNConv (COO SpMM aggregation + dense GEMM) on 8 Trainium2 NeuronCores.

  msgs = edge_vals[:, None] * x[edge_col]          # [E, 64] gather+scale
  agg  = segment_sum(msgs, edge_row, N)            # [N, 64] scatter-add
  out  = agg @ weight                              # [N, 64] GEMM

Sharding: destination-node sharding (each core owns a contiguous row slab and
all edges targeting it) -> zero collectives.

The throughput limit is SWDGE descriptor generation for the per-edge row
gather (~7.75 ns/descriptor + ~540 ns/call on the Q7s), so the host-side prep
minimizes padded gather slots:
  - x is split into 4 unequal quarters [23040,23040,23040,31232] (int16
    gather indices), sized so each (block, quarter) edge-group mean sits
    well below a multiple of 128.
  - each core's 12544 rows are bin-packed into 98 blocks of 128 rows,
    balancing all 4 per-quarter degree sums, so the max (block, quarter)
    group stays within [512,512,512,768] slots (vs 640 for equal quarters
    without packing).  The row permutation is undone on the host at the end.

Per core:
  - gpsimd.dma_gather pulls 64-float source rows (256B each) from HBM
  - one DVE tensor_tensor per (supergroup, quarter) scales msgs by edge_vals
  - one DVE tensor_tensor per dest block builds the one-hot
    oh[e, r] = (dest[e] == r) via iota-compare with a broadcast dest AP
  - TensorE per 128-edge chunk: psum[128 rows, 64] += oh.T @ msgs
  - per block: PE-transpose agg -> aggT, then outT[64,128] = W.T @ aggT
  - one contiguous [64, rows] output DMA; host scatters rows back.
"""

import os
import sys

import numpy as np

if "/opt/trn_rl_repo" not in sys.path:
    sys.path.insert(0, "/opt/trn_rl_repo")

# ---------------------------------------------------------------- constants
N = 100000
E = 1600000
D = 64
CORES = 8
RPC = 12544          # rows per core (8*12544 = 100352 >= N)
BLOCKS = RPC // 128  # 98 dest blocks per core
Q = 4
QS = np.array([0, 23040, 46080, 69120, 100352], dtype=np.int64)  # quarter bounds
CAPQ = np.array([512, 512, 512, 768], dtype=np.int64)  # packing targets
G = 7                # dest blocks per gather super-group (98 = 14*7)
NGROUPS = BLOCKS // G

LAST_EXEC_TIME_NS = None
_CACHE = {}


def _pack_rows(deg):
    """Assign RPC rows (deg: [RPC, 4] per-quarter degrees) to BLOCKS blocks
    of 128, balancing all 4 quarter sums.  Returns perm_local[pos] = row,
    where pos = block*128 + slot."""
    order = np.argsort(-deg.sum(1), kind="stable")
    cur = np.zeros((BLOCKS, Q), np.float64)
    capf = CAPQ.astype(np.float64)
    perm_local = np.empty(RPC, np.int64)
    for rnd in range(128):
        batch = order[rnd * BLOCKS : (rnd + 1) * BLOCKS]
        bscore = (deg[batch] / capf).max(1)
        bo = batch[np.argsort(-bscore, kind="stable")]
        load = (cur / capf).max(1)
        blko = np.argsort(load, kind="stable")
        cur[blko] += deg[bo]
        perm_local[blko * 128 + rnd] = bo
    return perm_local


# ---------------------------------------------------------------- host prep
def _prep(x, weight, edge_vals, edge_row, edge_col):
    e_row = np.asarray(edge_row, dtype=np.int64)
    e_col = np.asarray(edge_col, dtype=np.int64)
    ev = np.asarray(edge_vals, dtype=np.float32)
    x = np.asarray(x, dtype=np.float32)
    weight = np.asarray(weight, dtype=np.float32)
    ne = e_row.shape[0]
    NPAD = CORES * RPC

    qq = np.searchsorted(QS, e_col, side="right") - 1
    lidx = (e_col - QS[qq]).astype(np.int16)

    # per-row per-quarter degrees -> per-core packing permutation
    deg_flat = np.bincount(e_row * Q + qq, minlength=NPAD * Q).reshape(NPAD, Q)
    perm = np.empty((CORES, RPC), np.int64)      # perm[k, pos] = global row
    pos_of_row = np.empty(NPAD, np.int64)        # core-local position
    for k in range(CORES):
        pl = _pack_rows(deg_flat[k * RPC : (k + 1) * RPC])
        perm[k] = k * RPC + pl
        pos_of_row[perm[k]] = np.arange(RPC)

    core = e_row // RPC
    pos = pos_of_row[e_row]
    blk = pos // 128
    dest = (pos % 128).astype(np.float32)

    # group counts -> per-quarter chunk counts (global static)
    gkey = (core * BLOCKS + blk) * Q + qq
    counts = np.bincount(gkey, minlength=CORES * BLOCKS * Q)
    cmax = counts.reshape(CORES * BLOCKS, Q).max(axis=0)
    Cq = np.maximum(1, -(-cmax // 128))          # [Q] chunks per group
    SLq = Cq * 128
    SLOTSB = int(SLq.sum())                      # slots per block
    NCH = int(Cq.sum())                          # chunk-columns per block
    qslotoff = np.concatenate([[0], np.cumsum(SLq)[:-1]])

    order = np.argsort(gkey, kind="stable")
    NGK = CORES * BLOCKS * Q
    starts = np.zeros(NGK, np.int64)
    starts[1:] = np.cumsum(counts)[:-1]
    gsort = gkey[order]
    rank = np.arange(ne, dtype=np.int64) - starts[gsort]
    cb = gsort // Q
    qs = gsort % Q
    slot = cb * SLOTSB + qslotoff[qs] + rank

    NSLOT = CORES * BLOCKS * SLOTSB
    idx_flat = np.zeros(NSLOT, np.int16)          # pad gathers row 0
    dst_flat = np.full(NSLOT, -1.0, np.float32)   # pad never matches iota
    val_flat = np.zeros(NSLOT, np.float32)        # pad scales to 0
    idx_flat[slot] = lidx[order]
    dst_flat[slot] = dest[order]
    val_flat[slot] = ev[order]

    slots = idx_flat.reshape(CORES, NGROUPS, G, SLOTSB)
    dsts = dst_flat.reshape(CORES, NGROUPS, G, SLOTSB)
    vals = val_flat.reshape(CORES, NGROUPS, G, SLOTSB)

    # gather idx per call (g, q): [G*SLq] block-major; wrap to [128, ./16]
    gi_parts = []
    for q in range(Q):
        arr = slots[:, :, :, qslotoff[q] : qslotoff[q] + SLq[q]]
        arr = np.ascontiguousarray(arr).reshape(CORES, NGROUPS, G * int(SLq[q]))
        w16 = arr.reshape(CORES, NGROUPS, -1, 16)
        w16 = np.moveaxis(w16, 3, 2)             # [C, NGR, 16, CALLE/16]
        gi_parts.append(np.tile(w16, (1, 1, 8, 1)))
    gidx = np.ascontiguousarray(np.concatenate(gi_parts, axis=3))

    # dst/val in chunk-column layout [C, NGR, 128, G*NCH]:
    # column (lb, q, c) = lb*NCH + qchunkoff[q] + c
    def to_cols(a):
        parts = []
        for q in range(Q):
            seg = a[:, :, :, qslotoff[q] : qslotoff[q] + SLq[q]]
            parts.append(
                np.ascontiguousarray(seg).reshape(
                    CORES, NGROUPS, G, int(Cq[q]), 128
                )
            )
        cols = np.concatenate(parts, axis=3)      # [C, NGR, G, NCH, 128]
        cols = cols.reshape(CORES, NGROUPS, G * NCH, 128)
        return np.ascontiguousarray(np.moveaxis(cols, 3, 2))

    gdst = to_cols(dsts)
    gval = to_cols(vals)

    x_pad = np.zeros((int(QS[-1]), D), np.float32)
    x_pad[:N] = x
    irep = np.broadcast_to(np.arange(128, dtype=np.float32), (128, NCH, 128)).copy()
    ident = np.eye(128, dtype=np.float32)

    in_maps = []
    for k in range(CORES):
        in_maps.append(
            {
                "xq": x_pad,
                "w": np.ascontiguousarray(weight),
                "irep": irep,
                "ident": ident,
                "gidx": np.ascontiguousarray(gidx[k]),
                "gdst": gdst[k],
                "gval": gval[k],
            }
        )
    return in_maps, tuple(int(c) for c in Cq), perm


# ------------------------------------------------------------- bass program
def _build(Cq):
    import concourse.bacc as bacc
    import concourse.mybir as mybir
    import concourse.tile as tile

    f32 = mybir.dt.float32
    i16 = mybir.dt.int16
    SLq = [c * 128 for c in Cq]
    NCH = sum(Cq)
    qchunkoff = [0]
    for c in Cq[:-1]:
        qchunkoff.append(qchunkoff[-1] + c)
    CALLE = [G * s for s in SLq]
    off16 = [0]
    for c in CALLE:
        off16.append(off16[-1] + c // 16)
    TOT16 = off16[-1]

    nc = bacc.Bacc(
        "TRN2", target_bir_lowering=False, debug=False, num_devices=CORES
    )
    NX = int(QS[-1])
    x_d = nc.dram_tensor("xq", [NX, D], f32, kind="ExternalInput")
    w_d = nc.dram_tensor("w", [D, D], f32, kind="ExternalInput")
    irep_d = nc.dram_tensor("irep", [128, NCH, 128], f32, kind="ExternalInput")
    id_d = nc.dram_tensor("ident", [128, 128], f32, kind="ExternalInput")
    gidx_d = nc.dram_tensor("gidx", [NGROUPS, 128, TOT16], i16, kind="ExternalInput")
    gdst_d = nc.dram_tensor(
        "gdst", [NGROUPS, 128, G * NCH], f32, kind="ExternalInput"
    )
    gval_d = nc.dram_tensor(
        "gval", [NGROUPS, 128, G * NCH], f32, kind="ExternalInput"
    )
    outT_d = nc.dram_tensor("outT", [D, RPC], f32, kind="ExternalOutput")

    eq = mybir.AluOpType.is_equal
    mul = mybir.AluOpType.mult

    with tile.TileContext(nc) as tc:
        with (
            tc.tile_pool(name="const", bufs=1) as cpool,
            tc.tile_pool(name="io", bufs=3) as iopool,
            tc.tile_pool(name="vh", bufs=3) as vhpool,
            tc.tile_pool(name="sb", bufs=4) as sbpool,
            tc.tile_pool(name="outsb", bufs=1) as opool,
            tc.tile_pool(name="pa", bufs=3, space="PSUM") as papool,
            tc.tile_pool(name="pt", bufs=2, space="PSUM") as ptpool,
            tc.tile_pool(name="po", bufs=2, space="PSUM") as popool,
        ):
            w_sb = cpool.tile([D, D], f32, name="w_sb")
            irep_sb = cpool.tile([128, NCH, 128], f32, name="irep_sb")
            id_sb = cpool.tile([128, 128], f32, name="id_sb")
            outT_sb = opool.tile([D, RPC], f32, name="outT_sb")
            nc.sync.dma_start(out=w_sb[:], in_=w_d[:])
            nc.sync.dma_start(out=irep_sb[:], in_=irep_d[:])
            nc.sync.dma_start(out=id_sb[:], in_=id_d[:])

            # persistent double-buffered msgs tiles (gather fills every slot;
            # idx pads gather row 0, so contents are always finite)
            NB = 2
            msgs_t = [
                [
                    cpool.tile([128, G, Cq[q], D], f32, name=f"msgs{bi}_{q}")
                    for q in range(Q)
                ]
                for bi in range(NB)
            ]

            for g in range(NGROUPS):
                idx_t = iopool.tile([128, TOT16], i16, tag="idx", name=f"idx{g}")
                dst_t = iopool.tile([128, G * NCH], f32, tag="dst", name=f"dst{g}")
                val_t = iopool.tile([128, G * NCH], f32, tag="val", name=f"val{g}")
                nc.sync.dma_start(out=idx_t[:], in_=gidx_d[g])
                nc.sync.dma_start(out=dst_t[:], in_=gdst_d[g])
                nc.sync.dma_start(out=val_t[:], in_=gval_d[g])

                msgs = msgs_t[g % NB]
                for q in range(Q):
                    m = msgs[q]
                    nc.gpsimd.dma_gather(
                        m[:].rearrange("p g c d -> p (g c) d"),
                        x_d[int(QS[q]) : int(QS[q + 1]), :],
                        idx_t[:, off16[q] : off16[q + 1]],
                        CALLE[q],
                        CALLE[q],
                        D,
                        # single_packet=True needs the whole call inside the
                        # 1024-desc SWDGE ring -> device crash on big calls
                        single_packet=False,
                    )
                    # scale msgs by edge_vals (broadcast along features);
                    # val=0 pads zero the padded slots
                    nc.vector.tensor_tensor(
                        m[:],
                        m[:],
                        val_t[:]
                        .rearrange("p (l n) -> p l n", l=G)[
                            :, :, qchunkoff[q] : qchunkoff[q] + Cq[q]
                        ]
                        .unsqueeze(3)
                        .broadcast_to([128, G, Cq[q], D]),
                        mul,
                    )

                for lb in range(G):
                    b = g * G + lb
                    # one-hot for the whole block in one DVE op
                    vh = vhpool.tile([128, NCH, 128], f32, tag="vh", name=f"vh{b}")
                    nc.vector.tensor_tensor(
                        vh[:],
                        irep_sb[:],
                        dst_t[:, lb * NCH : (lb + 1) * NCH]
                        .unsqueeze(2)
                        .broadcast_to([128, NCH, 128]),
                        eq,
                    )
                    pa = papool.tile([128, D], f32, tag="pa", name=f"pa{b}")
                    nmm = NCH
                    i = 0
                    for q in range(Q):
                        for c in range(Cq[q]):
                            nc.tensor.matmul(
                                pa[:],
                                vh[:, qchunkoff[q] + c, :],
                                msgs[q][:, lb, c, :],
                                start=(i == 0),
                                stop=(i == nmm - 1),
                            )
                            i += 1
                    agg_sb = sbpool.tile([128, D], f32, tag="agg", name=f"agg{b}")
                    nc.vector.tensor_copy(agg_sb[:], pa[:])
                    pt = ptpool.tile([D, 128], f32, tag="pt", name=f"pt{b}")
                    nc.tensor.transpose(pt[:], agg_sb[:], id_sb[:])
                    aggT_sb = sbpool.tile([D, 128], f32, tag="aggT", name=f"aggT{b}")
                    nc.vector.tensor_copy(aggT_sb[:], pt[:])
                    po = popool.tile([D, 128], f32, tag="po", name=f"po{b}")
                    nc.tensor.matmul(po[:], w_sb[:], aggT_sb[:], start=True, stop=True)
                    nc.vector.tensor_copy(
                        outT_sb[:, b * 128 : (b + 1) * 128], po[:]
                    )

            nc.sync.dma_start(out=outT_d[:], in_=outT_sb[:])

    nc.compile()
    return nc


# ----------------------------------------------------------------- kernel()
def _ensure_ntff_hook():
    """Provide antenv.axon_hooks (absent in this image) so that
    run_bass_kernel_spmd's BASS_TRACE path can register the axon NTFF
    profiler instead of crashing on import."""
    try:
        import antenv.axon_hooks  # noqa: F401

        return
    except ImportError:
        pass
    import types

    import antenv

    mod = types.ModuleType("antenv.axon_hooks")
    holder = {"hook": None}
    mod.set_axon_ntff_profile_hook = lambda h: holder.__setitem__("hook", h)
    mod.get_axon_ntff_profile_hook = lambda: holder["hook"]
    sys.modules["antenv.axon_hooks"] = mod
    antenv.axon_hooks = mod
    try:
        from trn_agent_boot.trn_boot import _ntff_profile_via_ctypes

        mod.set_axon_ntff_profile_hook(
            _ntff_profile_via_ctypes("/opt/axon/libaxon_pjrt.so")
        )
    except Exception:
        pass


def kernel(x, weight, edge_vals, edge_row, edge_col):
    global LAST_EXEC_TIME_NS
    from concourse.bass_utils import run_bass_kernel_spmd

    if os.environ.get("BASS_TRACE"):
        _ensure_ntff_hook()

    in_maps, Cq, perm = _prep(x, weight, edge_vals, edge_row, edge_col)
    if Cq not in _CACHE:
        _CACHE[Cq] = _build(Cq)
    nc = _CACHE[Cq]

    res = run_bass_kernel_spmd(nc, in_maps, list(range(CORES)))
    LAST_EXEC_TIME_NS = res.exec_time_ns

    out = np.empty((CORES * RPC, D), np.float32)
    for k in range(CORES):
        out[perm[k]] = res.results[k]["outT"].T
    return np.ascontiguousarray(out[:N])
